# revision 3
# baseline (speedup 1.0000x reference)
"""BiRNN encoder-decoder Trainium2 kernel, v2 ("layout A").

Data-parallel over batch (8 cores x 16 rows). The hidden state lives
TRANSPOSED in SBUF as fp16: hT chunks [128 partitions = H-slice, 16 cols =
batch]. Every recurrence matmul is out[128,16] += W_block.T @ h_chunk with the
128x128 weight block stationary (f32r for the encoder, fp16 for the decoder)
and the fp16 state as the 16-wide moving operand (1 cyc/row in the PE).
tanh(psum)->SBUF-fp16 happens in ONE activation per step; biases are folded
into stationary blocks (extra ones-row / K=1 matmuls), so there are no
per-step DVE ops and no PE transposes at all.

Layer-1's input projection z1 = Wih1 @ [l0fwd; l0bwd] is batched over time as
dense N=512 GEMMs (f32r, 1 cyc/row) and interleaved with the L1 recurrence.
The decoder's autoregressive input feedback is algebraically collapsed: the
rank-1 path o0 -> next-input -> Wih0 becomes a dense C2 = v @ lin_W matmul
straight from h3, so the only cross-step serial chain is the four tanh
activations; the scalar head/input recursion runs off-chain on PE+DVE.
"""
import os
import numpy as np
from contextlib import ExitStack

import concourse.bacc as bacc
import concourse.tile as tile
from concourse import mybir
from concourse.bass_utils import run_bass_kernel_spmd

B, T, IN, H, TGT = 128, 128, 3, 512, 32
NC = 8
BC = B // NC          # 16 batch rows per core
CH = H // 128         # 4 chunks of the hidden dim
KB = 128
F32 = mybir.dt.float32
F32R = mybir.dt.float32r
FP16 = mybir.dt.float16
Tanh = mybir.ActivationFunctionType.Tanh
Ident = mybir.ActivationFunctionType.Identity

_prog_cache = {}

def _knob(name, default):
    return os.environ.get(name, default)

# scheduling knobs: "<count>x<n>" filler patterns, z1 drip rate/piece size
L0D = _knob("L0D", "0x128")
DECD = _knob("DECD", "0x256")
HEADD = _knob("HEADD", "0x256")
L1POP = _knob("L1POP", "23")  # digit cycle, e.g. "23" = alternate 2,3
Z1N = int(_knob("Z1N", "512"))
W0SPLIT = _knob("W0SPLIT", "0") == "1"
Z1STAGE = _knob("Z1STAGE", "0") == "1"

def _fill(pat, f):
    cnt, n = pat.split("x")
    for _ in range(int(cnt)):
        f(int(n))


def _build_program():
    if "nc" in _prog_cache:
        return _prog_cache["nc"]
    nc = bacc.Bacc("TRN2")
    dp = nc.declare_dram_parameter

    # --- dram params (host packs everything in tile layout) ---
    xT_e = dp("xT", [4, T * BC], FP16, isOutput=False)        # rows x0,x1,x2,ones; col t*16+b
    wxb0_e = dp("wxb0", [4, 2 * CH * KB], FP16, isOutput=False)  # (d,c): [Wih0_d.T rows; bias]
    whh0_e = dp("whh0", [128, 2 * CH * CH * KB], FP16, isOutput=False)
    whh1_e = dp("whh1", [128, 2 * CH * CH * KB], FP16, isOutput=False)
    wih1_e = dp("wih1", [128, 2 * CH * 8 * KB], FP16, isOutput=False)
    zbias_e = dp("zbias", [1, 2 * CH * KB], FP16, isOutput=False)
    ones_e = dp("ones", [1, 512], FP16, isOutput=False)
    ident_e = dp("ident", [128, 128], FP16, isOutput=False)
    dwhh_e = dp("dwhh", [128, 4 * CH * CH * KB], FP16, isOutput=False)
    dwihr_e = dp("dwihr", [128, 3 * CH * CH * KB], FP16, isOutput=False)
    c2_e = dp("c2", [128, CH * CH * KB], FP16, isOutput=False)
    dbias_e = dp("dbias", [1, 3 * CH * KB], FP16, isOutput=False)
    sinj0_e = dp("sinj0", [4, CH * KB], FP16, isOutput=False)
    sinj_e = dp("sinj", [3, CH * KB], FP16, isOutput=False)
    linw_e = dp("linw", [128, CH], FP16, isOutput=False)
    a1_e = dp("a1", [3, 3], FP16, isOutput=False)
    a2_e = dp("a2", [1, 3], F32, isOutput=False)
    linb_e = dp("linb", [1, 1], F32, isOutput=False)
    sT0_e = dp("sT0", [4, BC], FP16, isOutput=False)
    sT1_e = dp("sT1", [3, BC], FP16, isOutput=False)
    out_e = dp("out", [1, TGT * BC], F32, isOutput=True)

    with tile.TileContext(nc) as tc, ExitStack() as ctx:
        wp = ctx.enter_context(tc.tile_pool(name="w", bufs=1))
        hp = ctx.enter_context(tc.tile_pool(name="h", bufs=1))
        pp = ctx.enter_context(tc.tile_pool(name="ps", bufs=1, space="PSUM"))

        # --- SBUF weight tiles + loads (fast SP queue for L0-critical, Pool for bulk) ---
        xT = wp.tile([4, T * BC], FP16)
        wxb0 = wp.tile([4, 2 * CH * KB], FP16)
        whh0 = wp.tile([128, 2 * CH * CH * KB], FP16)
        nc.sync.dma_start(xT[:], xT_e[:])
        nc.sync.dma_start(wxb0[:], wxb0_e[:])
        if W0SPLIT:
            nc.sync.dma_start(whh0[:, 0:CH * CH * KB], whh0_e[:, 0:CH * CH * KB])
            nc.sync.dma_start(whh0[:, CH * CH * KB:], whh0_e[:, CH * CH * KB:])
        else:
            nc.sync.dma_start(whh0[:], whh0_e[:])
        whh1 = wp.tile([128, 2 * CH * CH * KB], FP16)
        wih1 = wp.tile([128, 2 * CH * 8 * KB], FP16)
        zbias = wp.tile([1, 2 * CH * KB], FP16)
        ones = wp.tile([1, 512], FP16)
        ident = wp.tile([128, 128], FP16)
        nc.gpsimd.dma_start(whh1[:], whh1_e[:])
        nc.gpsimd.dma_start(wih1[:], wih1_e[:])
        nc.gpsimd.dma_start(zbias[:], zbias_e[:])
        nc.gpsimd.dma_start(ones[:], ones_e[:])
        nc.gpsimd.dma_start(ident[:], ident_e[:])
        dwhh = wp.tile([128, 4 * CH * CH * KB], FP16)
        dwihr = wp.tile([128, 3 * CH * CH * KB], FP16)
        c2 = wp.tile([128, CH * CH * KB], FP16)
        dbias = wp.tile([1, 3 * CH * KB], FP16)
        sinj0 = wp.tile([4, CH * KB], FP16)
        sinj = wp.tile([3, CH * KB], FP16)
        linw = wp.tile([128, CH], FP16)
        a1 = wp.tile([3, 3], FP16)
        a2 = wp.tile([1, 3], F32)
        linb = wp.tile([1, 1], F32)
        st0 = wp.tile([4, BC], FP16)
        for t_, e_ in ((dwhh, dwhh_e), (dwihr, dwihr_e), (c2, c2_e), (dbias, dbias_e),
                       (sinj0, sinj0_e), (sinj, sinj_e), (linw, linw_e), (a1, a1_e),
                       (a2, a2_e), (linb, linb_e), (st0, sT0_e)):
            nc.gpsimd.dma_start(t_[:], e_[:])

        # --- persistent state stores ---
        l0s = [hp.tile([128, CH * T * BC], FP16, name=f"l0s{d}", tag=f"l0s{d}")
               for d in range(2)]
        z1s = [hp.tile([128, CH * T * BC], FP16, name=f"z1s{d}", tag=f"z1s{d}")
               for d in range(2)]
        outT = hp.tile([1, TGT * BC], F32, name="outT", tag="outT")

        mm = nc.tensor.matmul

        zq = []          # pending (deadline, op) z1 items

        def pop_q(k):
            out, zq[:k] = zq[:k], []
            for _, op_ in out:
                op_()
            return ()

        # ================= encoder layer 0 =================
        for t in range(T):
            for d in range(2):
                slot = t if d == 0 else T - 1 - t
                ps = pp.tile([128, CH * BC], F32, tag=f"ps{d}{t % 2}",
                             name="ps0", bufs=1)
                for c in range(CH):
                    mm(ps[:, BC * c:BC * (c + 1)], wxb0[:, (d * CH + c) * KB:(d * CH + c + 1) * KB],
                       xT[:, slot * BC:(slot + 1) * BC],
                       start=(c == 0), stop=(c == CH - 1))
                if t > 0:
                    pslot = t - 1 if d == 0 else T - t
                    for c in range(CH):
                        for k in range(CH):
                            mm(ps[:, BC * c:BC * (c + 1)],
                               whh0[:, ((d * CH + c) * CH + k) * KB:((d * CH + c) * CH + k + 1) * KB],
                               l0s[d][:, k * T * BC + pslot * BC: k * T * BC + (pslot + 1) * BC],
                               start=False, stop=(c == CH - 1 and k == CH - 1),
                               skip_group_check=True)
                nc.scalar.activation(
                    l0s[d][:].rearrange("p (k t) -> p k t", k=CH)[:, :, slot * BC:(slot + 1) * BC],
                    ps[:].rearrange("p (k b) -> p k b", k=CH), Tanh)
            if t == 111:
                for cc_ in range(CH):
                    zq += z1_chunk_ops(0, 16, 16 * BC, cc_)
                    zq += z1_chunk_ops(1, 96, 16 * BC, cc_)
            if Z1STAGE and t == 119:
                for cc_ in range(CH):
                    zq += z1_chunk_ops(0, 8, 8 * BC, cc_)
                    zq += z1_chunk_ops(1, 112, 8 * BC, cc_)
            if zq:
                pop_q(5 if not (Z1STAGE and t >= 119) else 8)
            elif 1 <= t < 112:
                _fill(L0D, dummy_mm)

        # ================= z1 = Wih1 @ [l0f; l0b] + bias1, batched over rt =================
        # Each (D, rt-range, c) chunk is one psum group: bias + 8 K-matmuls +
        # eviction. Emitted as closures popped a few at a time between
        # recurrence steps so the N-wide matmuls fill PE idle and keep the
        # PE p-state warm instead of bursting at mid-speed.
        def z1_chunk_ops(D, rt0, n, c):
            ops = []
            st = {}

            def open_group(D=D, rt0=rt0, n=n, c=c):
                st["psz"] = pp.tile([128, 512], F32, tag="psZ", name=f"z1_{D}_{rt0}_{c}",
                                    bufs=2)
                mm(st["psz"][:, 0:n], zbias[:, (D * CH + c) * KB:(D * CH + c + 1) * KB],
                   ones[0:1, 0:n], start=True, stop=False)

            ops.append(open_group)
            piece = min(Z1N, n)
            nh = n // piece
            for j in range(8):
                for h_ in range(nh):
                    def kmm(j=j, h_=h_, D=D, rt0=rt0, n=n, c=c, piece=piece, nh=nh):
                        srct = l0s[0] if j < 4 else l0s[1]
                        o0 = h_ * piece
                        mm(st["psz"][:, o0:o0 + piece],
                           wih1[:, ((D * CH + c) * 8 + j) * KB:((D * CH + c) * 8 + j + 1) * KB],
                           srct[:, (j % 4) * T * BC + rt0 * BC + o0: (j % 4) * T * BC + rt0 * BC + o0 + piece],
                           start=False, stop=(j == 7 and h_ == nh - 1))
                    ops.append(kmm)

            def evict(D=D, rt0=rt0, n=n, c=c):
                nc.vector.tensor_copy(
                    z1s[D][:, c * T * BC + rt0 * BC: c * T * BC + rt0 * BC + n],
                    st["psz"][:, 0:n])
            ops.append(evict)
            # all ops must be EMITTED before the first L1 step that consumes
            # this block (program order is the only write->read ordering the
            # framework can see)
            s_first = rt0 if D == 0 else T - 1 - (rt0 + n // BC - 1)
            return [(s_first, op_) for op_ in ops]

        def dummy_mm(n=512):
            # keeps the PE busy/warm through dependency stalls; result unused
            psx = pp.tile([128, 512], F32, tag="psZ", name="warm", bufs=2)
            mm(psx[:, 0:n], wxb0[:, 0:KB], xT[:, 0:n], start=True, stop=True)

        # drain stragglers, then the blocks gated by L0's very last steps
        pop_q(len(zq))
        nlate = 8 * BC if Z1STAGE else 16 * BC
        for cc_ in range(CH):
            for _, op in z1_chunk_ops(0, 0, nlate, cc_):
                op()
        for cc_ in range(CH):
            for _, op in z1_chunk_ops(1, 128 - nlate // BC, nlate, cc_):
                op()
        # remaining blocks, drip-fed through the L1 recurrence idle windows
        for b_ in range(3):
            for cc_ in range(CH):
                zq += z1_chunk_ops(0, 32 * (b_ + 1), 512, cc_)
                zq += z1_chunk_ops(1, 32 * (2 - b_), 512, cc_)

        # ================= encoder layer 1 =================
        h1 = [None, None]
        for s in range(T):
            b = s // 32
            while zq and zq[0][0] <= s:
                pop_q(1)
            for D in range(2):
                rt = s if D == 0 else T - 1 - s
                ps = pp.tile([128, CH * BC], F32, tag=f"ps{D}{s % 2}",
                             name="ps1", bufs=1)
                for c in range(CH):
                    mm(ps[:, BC * c:BC * (c + 1)], ident[:],
                       z1s[D][:, c * T * BC + rt * BC: c * T * BC + (rt + 1) * BC],
                       start=(c == 0), stop=(c == CH - 1))
                if s > 0:
                    for c in range(CH):
                        for k in range(CH):
                            mm(ps[:, BC * c:BC * (c + 1)],
                               whh1[:, ((D * CH + c) * CH + k) * KB:((D * CH + c) * CH + k + 1) * KB],
                               h1[D][:, BC * k:BC * (k + 1)],
                               start=False, stop=(c == CH - 1 and k == CH - 1),
                               skip_group_check=True)
                hn = hp.tile([128, CH * BC], FP16, tag=f"h1_{D}", name=f"h1_{D}", bufs=2)
                nc.scalar.activation(hn[:], ps[:], Tanh)
                h1[D] = hn
            # prefetch next z1 blocks, one (D,c) chunk every 4 super-steps
            if zq:
                pop_q(int(L1POP[s % len(L1POP)]))
            else:
                _fill(L0D, dummy_mm)

        # ================= decoder =================
        def l0s_ap(d, slot, k):
            return l0s[d][:, k * T * BC + slot * BC: k * T * BC + (slot + 1) * BC]

        h_ap = [
            (lambda k: l0s_ap(0, T - 1, k)),
            (lambda k: l0s_ap(1, 0, k)),
            (lambda k, h=h1[0]: h[:, BC * k:BC * (k + 1)]),
            (lambda k, h=h1[1]: h[:, BC * k:BC * (k + 1)]),
        ]
        sT_cur = hp.tile([3, BC], FP16, tag="sT", name="sT1", bufs=2)
        nc.sync.dma_start(sT_cur[:], sT1_e[:])
        h3_prev = None

        pso_prev = None
        for t in range(TGT):
            new_h = []
            # ---- "open" groups (bias + Whh terms; all gates at least one step
            # old) emitted ahead of the serial chain so they dispatch and run
            # inside the previous activation's latency window ----
            ps_l = {}
            for l in (1, 2):
                ps = pp.tile([128, CH * BC], F32,
                             tag=["ps10", "ps01", "ps11"][l - 1], name=f"psd{l}", bufs=1)
                for c in range(CH):
                    mm(ps[:, BC * c:BC * (c + 1)],
                       dbias[:, ((l - 1) * CH + c) * KB:((l - 1) * CH + c + 1) * KB],
                       ones[0:1, 0:BC], start=(c == 0), stop=False)
                for c in range(CH):
                    for k in range(CH):
                        mm(ps[:, BC * c:BC * (c + 1)],
                           dwhh[:, ((l * CH + c) * CH + k) * KB:((l * CH + c) * CH + k + 1) * KB],
                           h_ap[l](k), start=False,
                           stop=(c == CH - 1 and k == CH - 1))
                ps_l[l] = ps
            ps = pp.tile([128, CH * BC], F32, tag="ps00", name="psd0", bufs=1)
            if t == 0:
                for c in range(CH):
                    mm(ps[:, BC * c:BC * (c + 1)], sinj0[:, c * KB:(c + 1) * KB], st0[:],
                       start=(c == 0), stop=False)
            else:
                for c in range(CH):
                    mm(ps[:, BC * c:BC * (c + 1)], sinj[:, c * KB:(c + 1) * KB], sT_cur[:],
                       start=(c == 0), stop=False)
            for c in range(CH):
                for k in range(CH):
                    mm(ps[:, BC * c:BC * (c + 1)],
                       dwhh[:, (c * CH + k) * KB:(c * CH + k + 1) * KB], h_ap[0](k),
                       start=False, stop=(c == CH - 1 and k == CH - 1))
            # ---- on-chain: C2 @ h3_{t-1} closes layer 0's psum ----
            if t > 0:
                for c in range(CH):
                    for k in range(CH):
                        mm(ps[:, BC * c:BC * (c + 1)],
                           c2[:, (c * CH + k) * KB:(c * CH + k + 1) * KB],
                           h3_prev[:, BC * k:BC * (k + 1)],
                           start=False, stop=(c == CH - 1 and k == CH - 1),
                           skip_group_check=True)
                # head of step t-1 shares the same gate; runs in act0's window
                pso = pp.tile([1, BC], F32, tag="psO", name="psO")
                for k in range(CH):
                    mm(pso[:], linw[:, k:k + 1], h3_prev[:, BC * k:BC * (k + 1)],
                       start=(k == 0), stop=(k == CH - 1))
                pso_prev = pso
            h0 = hp.tile([128, CH * BC], FP16, tag="hd0", name="hd0", bufs=2)
            nc.scalar.activation(h0[:], ps[:], Tanh)
            new_h.append(h0)
            # open for layer 3 (gate = act3(t-1)+eff, fires at the step start)
            ps = pp.tile([128, CH * BC], F32, tag="ps11", name="psd3", bufs=1)
            for c in range(CH):
                mm(ps[:, BC * c:BC * (c + 1)],
                   dbias[:, (2 * CH + c) * KB:(2 * CH + c + 1) * KB],
                   ones[0:1, 0:BC], start=(c == 0), stop=False)
            for c in range(CH):
                for k in range(CH):
                    mm(ps[:, BC * c:BC * (c + 1)],
                       dwhh[:, ((3 * CH + c) * CH + k) * KB:((3 * CH + c) * CH + k + 1) * KB],
                       h_ap[3](k), start=False, stop=(c == CH - 1 and k == CH - 1))
            ps_l[3] = ps
            if t > 0:
                nc.vector.tensor_scalar_add(outT[0:1, (t - 1) * BC:t * BC], pso_prev[:],
                                            linb[0:1, 0:1])

            # ---- layers 1..3: only the input-from-below matmuls are on-chain ----
            for l in range(1, 4):
                ps = ps_l[l]
                for c in range(CH):
                    for k in range(CH):
                        mm(ps[:, BC * c:BC * (c + 1)],
                           dwihr[:, (((l - 1) * CH + c) * CH + k) * KB:(((l - 1) * CH + c) * CH + k + 1) * KB],
                           new_h[l - 1][:, BC * k:BC * (k + 1)],
                           start=False, stop=(c == CH - 1 and k == CH - 1),
                           skip_group_check=True)
                hl = hp.tile([128, CH * BC], FP16, tag=f"hd{l}", name=f"hd{l}", bufs=2)
                nc.scalar.activation(hl[:], ps[:], Tanh)
                new_h.append(hl)
                # off-chain sT recursion for step t+1
                if l == 1 and 1 <= t < TGT - 1:
                    pss = pp.tile([3, BC], F32, tag="psS", name="psS")
                    mm(pss[:], a1[:], sT_cur[:], start=True, stop=False)
                    mm(pss[:], a2[:], outT[0:1, (t - 1) * BC:t * BC], start=False, stop=True)
                    sT_new = hp.tile([3, BC], FP16, tag="sT", name="sTn", bufs=2)
                    nc.vector.tensor_copy(sT_new[:], pss[:])
                    sT_cur = sT_new

            h3_prev = new_h[3]
            h_ap = [(lambda k, h=hh: h[:, BC * k:BC * (k + 1)]) for hh in new_h]

        # final head (step TGT-1)
        pso = pp.tile([1, BC], F32, tag="psO", name="psO")
        for k in range(CH):
            mm(pso[:], linw[:, k:k + 1], h3_prev[:, BC * k:BC * (k + 1)],
               start=(k == 0), stop=(k == CH - 1))
        nc.vector.tensor_scalar_add(outT[0:1, (TGT - 1) * BC:TGT * BC], pso[:],
                                    linb[0:1, 0:1])

        nc.sync.dma_start(out_e[:], outT[:])

    nc.compile()
    _prog_cache["nc"] = nc
    return nc


def _pack_blocksT(M):
    """M [I, J] (I=out, J=in, both multiples of 128) -> [128, (I/128)*(J/128)*128]
    with stationary lhsT block (c,k) = M.T[128k:128k+128, 128c:128c+128] at
    col ((c*nk + k)*128)."""
    I, J = M.shape
    ncb, nkb = I // 128, J // 128
    A = M.reshape(ncb, 128, nkb, 128)   # [c, i, k, p]
    return np.ascontiguousarray(A.transpose(3, 0, 2, 1).reshape(128, ncb * nkb * 128))


def kernel(x, y, enc_Wih0, enc_Whh0, enc_Wih1, enc_Whh1, enc_bih, enc_bhh,
           dec_Wih0, dec_Wihr, dec_Whh, dec_bih, dec_bhh, lin_W, lin_b,
           target_len, teacher_forcing_ratio):
    f = np.float32
    h = np.float16
    x = np.asarray(x, f)
    enc_Wih0 = np.asarray(enc_Wih0, f); enc_Whh0 = np.asarray(enc_Whh0, f)
    enc_Wih1 = np.asarray(enc_Wih1, f); enc_Whh1 = np.asarray(enc_Whh1, f)
    enc_bih = np.asarray(enc_bih, f); enc_bhh = np.asarray(enc_bhh, f)
    dec_Wih0 = np.asarray(dec_Wih0, f); dec_Wihr = np.asarray(dec_Wihr, f)
    dec_Whh = np.asarray(dec_Whh, f); dec_bih = np.asarray(dec_bih, f)
    dec_bhh = np.asarray(dec_bhh, f)
    lin_W = np.asarray(lin_W, f); lin_b = np.asarray(lin_b, f).reshape(())

    # ---- shared (batch-independent) weight packs ----
    wxb0 = np.zeros((4, 2 * CH * KB), h)
    for d in range(2):
        b0 = enc_bih[0, d] + enc_bhh[0, d]
        for c in range(CH):
            wxb0[:3, (d * CH + c) * KB:(d * CH + c + 1) * KB] = enc_Wih0[d].T[:, c * KB:(c + 1) * KB]
            wxb0[3, (d * CH + c) * KB:(d * CH + c + 1) * KB] = b0[c * KB:(c + 1) * KB]
    whh0 = np.concatenate([_pack_blocksT(enc_Whh0[d]) for d in range(2)], 1).astype(h)
    whh1 = np.concatenate([_pack_blocksT(enc_Whh1[d]) for d in range(2)], 1).astype(h)
    wih1 = np.concatenate([_pack_blocksT(enc_Wih1[d]) for d in range(2)], 1).astype(h)
    zbias = np.concatenate([(enc_bih[1, d] + enc_bhh[1, d]) for d in range(2)])[None, :].astype(h)
    ones = np.ones((1, 512), h)
    ident = np.eye(128, dtype=h)
    dwhh = np.concatenate([_pack_blocksT(dec_Whh[l]) for l in range(4)], 1).astype(h)
    dwihr = np.concatenate([_pack_blocksT(dec_Wihr[l]) for l in range(3)], 1).astype(h)
    v = dec_Wih0[:, 0] - dec_Wih0[:, 1] + dec_Wih0[:, 2]
    c2m = _pack_blocksT(np.outer(v, lin_W[0])).astype(h)
    db0 = dec_bih[0] + dec_bhh[0]
    dbias = np.concatenate([dec_bih[l] + dec_bhh[l] for l in range(1, 4)])[None, :].astype(h)
    sinj0 = np.zeros((4, CH * KB), f)
    sinj0[:3] = dec_Wih0.T
    sinj0[3] = db0
    sinj = np.zeros((3, CH * KB), f)
    sinj[0] = dec_Wih0[:, 1] - dec_Wih0[:, 2]
    sinj[1] = dec_Wih0[:, 2]
    sinj[2] = db0 + v * float(lin_b)
    linw = lin_W[0].reshape(CH, KB).T.astype(h)   # [128, 4], col c = lin_W chunk c
    a1 = np.zeros((3, 3), h); a1[0, 1] = 1.0; a1[2, 2] = 1.0
    a2 = np.array([[1.0, -1.0, 0.0]], f)
    linb = np.array([[float(lin_b)]], f)

    shared = {
        "wxb0": wxb0, "whh0": whh0, "whh1": whh1, "wih1": wih1,
        "zbias": np.ascontiguousarray(zbias), "ones": ones, "ident": ident,
        "dwhh": dwhh, "dwihr": dwihr, "c2": c2m,
        "dbias": np.ascontiguousarray(dbias), "sinj0": sinj0.astype(h),
        "sinj": sinj.astype(h), "linw": np.ascontiguousarray(linw),
        "a1": a1, "a2": a2, "linb": linb,
    }

    nc = _build_program()
    in_maps = []
    for cc in range(NC):
        xc = x[cc * BC:(cc + 1) * BC]          # (16, T, 3)
        xT = np.empty((4, T * BC), h)
        xT[:3] = xc.transpose(2, 1, 0).reshape(3, T * BC)
        xT[3] = 1.0
        sT0 = np.empty((4, BC), h)
        sT0[:3] = xc[:, -1, :].T
        sT0[3] = 1.0
        sT1 = np.empty((3, BC), h)
        sT1[:2] = xc[:, -1, :2].T
        sT1[2] = 1.0
        m = dict(shared)
        m["xT"] = xT
        m["sT0"] = sT0
        m["sT1"] = sT1
        in_maps.append(m)
    res = run_bass_kernel_spmd(nc, in_maps, list(range(NC)))
    out = np.stack([res.results[cc]["out"].reshape(TGT, BC).T for cc in range(NC)])
    return out.reshape(B, TGT, 1).astype(np.float32)
